# revision 18
# baseline (speedup 1.0000x reference)
"""Trainium2 Bass kernel for CRF negative-log-likelihood loss (v5).

Problem: nn_CRF (B=512, L=1024, T=48), data-parallel over 8 NeuronCores
(64 batch rows per core); host sums the 8 partial losses.

Design:
  - Bidirectional exact scan: forward chain (t=0..511) and backward
    adjoint chain (t=1023..512) run concurrently, stacked on partitions
    0-47 / 64-111 of one (128, 64) state tile. 512 serial steps, each =
    ONE bf16 (128x128)-stationary matmul + ONE 128-partition DVE
    scalar_tensor_tensor: a_cur = max(ps, svec) * F.
  - Variable lengths (all >= L/2, so the junction at t=511 is live):
    the backward chain starts from zero state; exp(end) is injected at
    t = len_b through stationary row 112 driven by an indicator row
    ind_k (1 iff len_b == 1024-k) that rides at partition 112 of the
    state. The Hadamard regenerates it each step: F tiles carry
    ind_{k+1} in row 112 (tiny DMA from a host-packed stream), svec =
    e_112, and stationary column 112 is zero, so max(0,1)*ind_{k+1}
    rewrites row 112 while live rows (nonnegative) pass through
    max(ps, 0) untouched. Z_b = alpha_511 . beta_511 at the junction.
  - No renormalization (drift ~ e^+-15 over 512 steps, validated).
  - feats converted to bf16 on host: halves HBM traffic, 1-cycle/row
    PE transposes.
  - Gold trans/start/end terms via one gpsimd ap_gather over a
    host-packed index tensor (masked pairs hit a zero table slot).
  - Gold feat term reuses the raw transposed feat tiles of the scan's
    F-prep (no second HBM read): per 8-step window, a ones-row matmul
    broadcasts a streamed masked-tag row to 48 partitions in PSUM, and
    one fused is_equal/mult/accumulate DVE op per direction dots the
    one-hot with the raw feats (ACT copies them to SBUF alongside the
    exp for the scan).
"""

import math

import numpy as np
import ml_dtypes

import concourse.bacc as bacc
import concourse.mybir as mybir
import concourse.tile as tile
from concourse.bass_utils import run_bass_kernel_spmd
from concourse.hw_specs import TRN2Spec

# The default cost model credits gpsimd ISA ops with 0.6x-roofline
# throughput; the real ap_gather ucode runs ~2 cycles per (channel,
# idx) element (~216us for this kernel's 8208x16-per-core gather).
# The Tile scheduler uses the cost model to order the per-engine
# queues, and with the optimistic default it slots the gather's
# consumers mid-scan, where they block the in-order PE queue for the
# gather's true duration. Calibrating the efficiency makes the
# scheduler place everything that depends on the gather after the
# scan, so the gather overlaps the scan instead of stalling it.
TRN2Spec.GPSIMD_IMPL_EFFICIENCY = {
    **TRN2Spec.GPSIMD_IMPL_EFFICIENCY, "APGather": 0.03}

F32 = mybir.dt.float32
BF16 = mybir.dt.bfloat16
I16 = mybir.dt.int16
I32 = mybir.dt.int32
AF = mybir.ActivationFunctionType
OP = mybir.AluOpType

B_FULL = 512
N_CORES = 8
BC = B_FULL // N_CORES          # 64
L_FULL = 1024
T = 48
MID = L_FULL // 2               # 512 junction
MU = 0.51
ASH = math.log(T)

FCH = 32                        # timesteps per natf chunk DMA
WIN = 8                         # steps per F tile window
NWIN = MID // WIN               # 64
NCH = MID // FCH                # 16 chunks per direction

NIDX = 1025                     # gather idxs per b: 1023 trans + start + end
NIDX_CORE = 8208                # 8*1025 rounded up to %16==0 (pad 2304)
TBL = 2401                      # 2304 trans + zero + 48 start + 48 end


def build_program(dbg=False):
    nc = bacc.Bacc("TRN2", target_bir_lowering=False, debug=False)

    feats_d = nc.dram_tensor("feats", (BC, L_FULL, T), BF16,
                             kind="ExternalInput")
    w128_d = nc.dram_tensor("w128", (128, 128), BF16, kind="ExternalInput")
    ident_d = nc.dram_tensor("ident", (64, 64), BF16, kind="ExternalInput")
    ainit_d = nc.dram_tensor("ainit", (128, BC), BF16, kind="ExternalInput")
    svec_d = nc.dram_tensor("svec", (128, 1), F32, kind="ExternalInput")
    indf_d = nc.dram_tensor("indf", (1, NWIN * 512), BF16,
                            kind="ExternalInput")
    tagwf_d = nc.dram_tensor("tagwf", (1, NWIN * 512), BF16,
                             kind="ExternalInput")
    tagwb_d = nc.dram_tensor("tagwb", (1, NWIN * 512), BF16,
                             kind="ExternalInput")
    idx_d = nc.dram_tensor("idxw", (128, NIDX_CORE // 16), I16,
                           kind="ExternalInput")
    tbl_d = nc.dram_tensor("tbl", (1, TBL), F32, kind="ExternalInput")
    shifts_d = nc.dram_tensor("shifts", (1, BC), F32, kind="ExternalInput")
    colsel_d = nc.dram_tensor("colsel", (128, 1), F32, kind="ExternalInput")
    out_d = nc.dram_tensor("out", (1, 1), F32, kind="ExternalOutput")
    dbg_d = (nc.dram_tensor("dbg", (4, BC), F32, kind="ExternalOutput")
             if dbg else None)

    feats_flat = feats_d.ap().rearrange("b l t -> b (l t)")

    with tile.TileContext(nc) as tc:
        with (
            tc.tile_pool(name="const", bufs=1) as cp,
            tc.tile_pool(name="natfp", bufs=3) as natp,
            tc.tile_pool(name="natbp", bufs=3) as natbp,
            tc.tile_pool(name="tagp", bufs=3) as tagp,
            tc.tile_pool(name="ap", bufs=3) as apool,
            tc.tile_pool(name="selp", bufs=2) as selp,
            tc.tile_pool(name="tpfps", bufs=2, space="PSUM") as tpfp,
            tc.tile_pool(name="tpbps", bufs=2, space="PSUM") as tpbp,
            tc.tile_pool(name="tagps", bufs=1, space="PSUM") as tagpp,
            tc.tile_pool(name="scanps", bufs=2, space="PSUM") as scanp,
            tc.tile_pool(name="gps", bufs=1, space="PSUM") as gpsp,
        ):
            # ---------------- constants / params ----------------
            identM = cp.tile((64, 64), BF16)
            nc.sync.dma_start(identM[:, :], ident_d.ap())

            iotaPi = cp.tile((128, 1), I32)
            nc.gpsimd.iota(iotaPi[:, :], [[1, 1]], channel_multiplier=1)
            iotaPf = cp.tile((128, 1), F32)
            nc.vector.tensor_copy(iotaPf[:, :], iotaPi[:, :])

            bias_mu = cp.tile((T, 1), F32)
            nc.vector.memset(bias_mu[:, :], -MU)
            ones48c = cp.tile((T, 1), F32)
            nc.vector.memset(ones48c[:, :], 1.0)
            ones1row = cp.tile((1, T), BF16)
            nc.vector.memset(ones1row[:, :], 1.0)
            colsel = cp.tile((128, 1), F32)
            nc.sync.dma_start(colsel[:, :], colsel_d.ap())
            svec = cp.tile((128, 1), F32)
            nc.sync.dma_start(svec[:, :], svec_d.ap())

            w128 = cp.tile((128, 128), BF16)
            nc.sync.dma_start(w128[:, :], w128_d.ap())
            idxw = cp.tile((128, NIDX_CORE // 16), I16)
            nc.sync.dma_start(idxw[:, :], idx_d.ap())
            tbl = cp.tile((128, TBL), F32)
            nc.sync.dma_start(tbl[:, :], tbl_d.ap().partition_broadcast(128))
            shifts = cp.tile((1, BC), F32)
            nc.sync.dma_start(shifts[:, :], shifts_d.ap())

            # F tiles: 3 persistent buffers, gap rows zeroed once
            fbufs = []
            for i in range(3):
                fb = cp.tile((128, 512), BF16, name=f"fbuf{i}")
                nc.vector.memset(fb[:, :], 0.0)
                fbufs.append(fb)
            rawf = cp.tile((T, 512), BF16, name="rawf")
            rawb = cp.tile((T, 512), BF16, name="rawb")

            a_init = cp.tile((128, BC), BF16)
            nc.sync.dma_start(a_init[:, :], ainit_d.ap())

            gout = cp.tile((128, NIDX_CORE), F32)
            feat_acc = cp.tile((T, 2 * NWIN), F32)

            gsum_ps = gpsp.tile((1, 512), F32, name="gsum")

            # ---------------- helper emitters ----------------
            natf_tiles = {}
            natb_tiles = {}

            def emit_chunk(c, bwd):
                pool = natbp if bwd else natp
                tl = pool.tile((BC, FCH * T), BF16,
                               name="natb" if bwd else "natf")
                if bwd:
                    lo = (L_FULL - FCH * (c + 1)) * T
                else:
                    lo = FCH * c * T
                nc.sync.dma_start(tl[:, :], feats_flat[:, lo:lo + FCH * T])
                (natb_tiles if bwd else natf_tiles)[c] = tl

            tag_tiles = {}

            def emit_tagw(m):
                tf = tagp.tile((1, 512), BF16, name="tagwf")
                nc.sync.dma_start(tf[:, :],
                                  tagwf_d.ap()[:, 512 * m:512 * (m + 1)])
                tb = tagp.tile((1, 512), BF16, name="tagwb")
                nc.sync.dma_start(tb[:, :],
                                  tagwb_d.ap()[:, 512 * m:512 * (m + 1)])
                tag_tiles[m] = (tf, tb)

            tp_tiles = {}

            def emit_tp(m, q):
                """Two transposes (fwd+bwd) for window m, col q."""
                if q == 0:
                    tpf = tpfp.tile((T, 512), BF16, name="tpf")
                    tpb = tpbp.tile((T, 512), BF16, name="tpb")
                    tp_tiles[m] = (tpf, tpb)
                tpf, tpb = tp_tiles[m]
                cf = m // 4
                colf = 8 * (m % 4) + q
                colb = 31 - 8 * (m % 4) - q
                nf = natf_tiles[cf]
                nb = natb_tiles[cf]
                nc.tensor.matmul(
                    tpf[:, 64 * q:64 * q + BC],
                    nf[:, T * colf:T * (colf + 1)],
                    identM[:, :], is_transpose=True, start=True,
                    stop=True, skip_group_check=True)
                nc.tensor.matmul(
                    tpb[:, 64 * q:64 * q + BC],
                    nb[:, T * colb:T * (colb + 1)],
                    identM[:, :], is_transpose=True, start=True,
                    stop=True, skip_group_check=True)

            def emit_fassm(m):
                """Assemble F tiles + gold-feat work for window m."""
                tpf, tpb = tp_tiles[m]
                fb = fbufs[m % 3]
                nc.scalar.activation(fb[0:T, :], tpf[:, :], AF.Exp,
                                     bias=bias_mu[:, :])
                nc.scalar.activation(fb[64:64 + T, :], tpb[:, :], AF.Exp,
                                     bias=bias_mu[:, :])
                nc.sync.dma_start(fb[112:113, :],
                                  indf_d.ap()[:, 512 * m:512 * (m + 1)])
                # raw copies for the gold-feat select
                nc.scalar.copy(rawf[:, :], tpf[:, :])
                nc.scalar.copy(rawb[:, :], tpb[:, :])
                tf, tb = tag_tiles[m]
                tbc = tagpp.tile((T, 512), F32, name="tagbc")
                nc.tensor.matmul(tbc[:, :], ones1row[:, :], tf[:, :],
                                 start=True, stop=True,
                                 skip_group_check=True)
                sel = selp.tile((T, 512), BF16, name="sel")
                nc.vector.scalar_tensor_tensor(
                    sel[:, :], tbc[:, :], iotaPf[0:T, :], rawf[:, :],
                    OP.is_equal, OP.mult,
                    accum_out=feat_acc[:, m:m + 1])
                tbc2 = tagpp.tile((T, 512), F32, name="tagbc")
                nc.tensor.matmul(tbc2[:, :], ones1row[:, :], tb[:, :],
                                 start=True, stop=True,
                                 skip_group_check=True)
                sel2 = selp.tile((T, 512), BF16, name="sel")
                nc.vector.scalar_tensor_tensor(
                    sel2[:, :], tbc2[:, :], iotaPf[0:T, :], rawb[:, :],
                    OP.is_equal, OP.mult,
                    accum_out=feat_acc[:, NWIN + m:NWIN + m + 1])

            # ---------------- pipeline ----------------
            emit_chunk(0, False)
            emit_chunk(0, True)
            emit_chunk(1, False)
            emit_chunk(1, True)
            emit_tagw(0)
            emit_tagw(1)
            for q in range(WIN):
                emit_tp(0, q)
            for q in range(WIN):
                emit_tp(1, q)
            emit_fassm(0)
            emit_fassm(1)

            a_prev = a_init
            jn = None

            for k in range(MID + 1):
                m = k // WIN
                q = k % WIN
                if q == 0 and k < MID:
                    cnext = m // 4 + 2
                    if m % 4 == 0 and cnext < NCH:
                        emit_chunk(cnext, False)
                        emit_chunk(cnext, True)
                    if m + 2 < NWIN:
                        emit_tagw(m + 2)
                if k < MID and m + 2 < NWIN:
                    emit_tp(m + 2, q)          # spread transposes
                    if q == WIN - 1:
                        emit_fassm(m + 2)

                ps = scanp.tile((128, BC), F32, name="ps")
                nc.tensor.matmul(ps[:, :], w128[:, :], a_prev[:, :],
                                 start=True, stop=True,
                                 skip_group_check=True)

                if k < MID:
                    fb = fbufs[m % 3]
                    a_cur = apool.tile((128, BC), BF16, name="a_t")
                    nc.vector.scalar_tensor_tensor(
                        a_cur[:, :], ps[:, :], svec[:, :],
                        fb[:, 64 * q:64 * (q + 1)],
                        OP.max, OP.mult)
                    a_prev = a_cur
                else:
                    jn = cp.tile((T, BC), F32)
                    nc.vector.tensor_tensor(jn[:, :], ps[64:64 + T, :],
                                            a_prev[0:T, :], OP.mult)

            # ---------------- gather (gpsimd, end of its queue) --------
            nc.gpsimd.ap_gather(gout[:, :], tbl[:, :], idxw[:, :],
                                channels=128, num_elems=TBL, d=1,
                                num_idxs=NIDX_CORE)
            for s in range(16):
                nc.tensor.matmul(gsum_ps[:, :], colsel[:, :],
                                 gout[:, 512 * s:512 * (s + 1)],
                                 start=(s == 0), stop=False,
                                 skip_group_check=True)
            nc.tensor.matmul(gsum_ps[:, 0:16], colsel[:, :],
                             gout[:, 8192:8208],
                             start=False, stop=True,
                             skip_group_check=True)

            # ---------------- end phase ----------------
            end_ps = tagpp.tile((T, 512), F32, name="tagbc")
            zps = end_ps[0:1, 0:BC]
            nc.tensor.matmul(zps, ones48c[:, :], jn[:, :],
                             start=True, stop=True, skip_group_check=True)
            lnz = cp.tile((1, BC), F32)
            nc.scalar.activation(lnz[:, :], zps, AF.Ln)
            fwdrow = cp.tile((1, BC), F32)
            nc.vector.tensor_tensor(fwdrow[:, :], lnz[:, :], shifts[:, :],
                                    OP.add)
            fwd_tot = cp.tile((1, 1), F32)
            nc.vector.tensor_reduce(fwd_tot[:, :], fwdrow[:, :],
                                    mybir.AxisListType.X, OP.add)

            gtr = cp.tile((1, 1), F32)
            nc.vector.tensor_reduce(gtr[:, :], gsum_ps[:, :],
                                    mybir.AxisListType.X, OP.add)
            fred = cp.tile((T, 1), F32)
            nc.vector.tensor_reduce(fred[:, :], feat_acc[:, :],
                                    mybir.AxisListType.X, OP.add)
            gfe_ps = end_ps[0:1, 256:257]
            nc.tensor.matmul(gfe_ps, fred[:, :], ones48c[:, :],
                             start=True, stop=True, skip_group_check=True)
            loss = cp.tile((1, 1), F32)
            nc.vector.tensor_tensor(loss[:, :], fwd_tot[:, :], gtr[:, :],
                                    OP.subtract)
            nc.vector.tensor_tensor(loss[:, :], loss[:, :], gfe_ps,
                                    OP.subtract)
            nc.sync.dma_start(out_d.ap(), loss[:, :])
            if dbg:
                nc.sync.dma_start(dbg_d.ap()[0:1, :], lnz[:, :])
                nc.sync.dma_start(dbg_d.ap()[1:2, :], fwdrow[:, :])

    nc.compile()
    return nc


def shard_inputs(feats, transitions, start_transitions, end_transitions,
                 tags, mask, n_cores=N_CORES):
    feats = np.asarray(feats, dtype=np.float32)
    trans = np.asarray(transitions, dtype=np.float32)
    start = np.asarray(start_transitions, dtype=np.float32)
    end = np.asarray(end_transitions, dtype=np.float32)
    tags = np.asarray(tags).astype(np.int64)
    mask = np.asarray(mask).astype(np.int64)
    B, L = tags.shape
    lens = mask.sum(1).astype(np.int64)              # (B,)
    feats16 = np.ascontiguousarray(feats.astype(ml_dtypes.bfloat16))

    E = np.exp(trans - ASH)
    w128 = np.zeros((128, 128), dtype=np.float32)
    w128[0:T, 0:T] = E                    # fwd: out_j = sum_i E[i,j] a_i
    w128[49, 0:T] = np.exp(start)         # fwd init via ones row
    w128[64:64 + T, 64:64 + T] = E.T      # bwd: out_i = sum_j E[i,j] g_j
    w128[112, 64:64 + T] = np.exp(end)    # inject row
    w128_16 = w128.astype(ml_dtypes.bfloat16)
    ident16 = np.eye(64, dtype=np.float32).astype(ml_dtypes.bfloat16)

    svec = np.zeros((128, 1), dtype=np.float32)
    svec[112, 0] = 1.0

    tbl = np.zeros((1, TBL), dtype=np.float32)
    tbl[0, 0:2304] = trans.reshape(-1)
    tbl[0, 2305:2305 + T] = start
    tbl[0, 2353:2353 + T] = end

    tagm_full = np.where(mask > 0, tags, 300).astype(np.float32)  # (B, L)

    in_maps = []
    for cidx in range(n_cores):
        sl = slice(cidx * BC, (cidx + 1) * BC)
        tg = tags[sl]
        mk = mask[sl]
        ln = lens[sl]
        tm = tagm_full[sl]                            # (BC, L)

        # F-tile indicator rows: window m col 64q+b = (len_b == 1023-8m-q)
        ks = np.arange(1, MID + 1)                    # ind_{k+1} for k=8m+q
        indf = (ln[None, :] == (L - ks)[:, None]).astype(np.float32)
        indf = np.ascontiguousarray(
            indf.reshape(1, -1)).astype(ml_dtypes.bfloat16)

        # tag window rows: [0, 512m + 64q + b] = masked tag at (q, b);
        # fwd t = 8m+q, bwd t = 1023-8m-q
        tagwf = np.ascontiguousarray(
            tm[:, 0:MID].T.reshape(1, -1)).astype(ml_dtypes.bfloat16)
        tagwb = np.ascontiguousarray(
            tm[:, MID:][:, ::-1].T.reshape(1, -1)).astype(
                ml_dtypes.bfloat16)

        ainit = np.zeros((128, BC), dtype=np.float32)
        ainit[112, :] = (ln == L)                     # ind_0
        ainit[49, :] = 1.0                            # fwd init ones row
        ainit16 = ainit.astype(ml_dtypes.bfloat16)

        idxw = np.full((128, NIDX_CORE // 16), 2304, dtype=np.int16)
        for g in range(8):
            lst = []
            for bb in range(8):
                b = 8 * g + bb
                v = (tg[b, :-1] * T + tg[b, 1:]).astype(np.int64)
                lst.append(v[mk[b, 1:] > 0])          # valid pairs only
                lst.append([2305 + tg[b, 0],
                            2353 + tg[b, ln[b] - 1]])
            flat = np.concatenate([np.asarray(x, dtype=np.int64)
                                   for x in lst])
            assert flat.size <= NIDX_CORE
            kk = np.arange(flat.size)
            idxw[16 * g + (kk % 16), kk // 16] = flat
        shifts = ((ln - 1) * ASH + ln * MU).astype(np.float32)[None, :]
        colsel = ((np.arange(128) % 16) == 0).astype(np.float32)[:, None]

        in_maps.append({
            "feats": feats16[sl],
            "w128": w128_16,
            "ident": ident16,
            "ainit": ainit16,
            "svec": svec,
            "indf": indf,
            "tagwf": tagwf,
            "tagwb": tagwb,
            "idxw": idxw,
            "tbl": tbl,
            "shifts": shifts,
            "colsel": colsel,
        })
    return in_maps


def kernel(feats, transitions, start_transitions, end_transitions, tags,
           mask, **_ignored):
    in_maps = shard_inputs(feats, transitions, start_transitions,
                           end_transitions, tags, mask)
    nc = build_program()
    res = run_bass_kernel_spmd(nc, in_maps, core_ids=list(range(N_CORES)))
    total = sum(float(r["out"][0, 0]) for r in res.results)
    return np.float32(total)
